# revision 3
# baseline (speedup 1.0000x reference)
"""Catmull-Rom spline loss kernel for Trainium2 (8 NeuronCores, SPMD) — v2.

loss = sum((ch1 - mapped)^2), mapped[n,c] = sum_{k,t} Wx[n,k] Wy[n,t]
CP_locs[i-1+k, j-1+t, c].

v1 used one 256B dma_gather descriptor per point; GpSimd software
descriptor generation runs at ~8ns/descriptor, capping 250K points/core
at ~2ms. v2 replaces the per-point gather with a one-hot matmul gather:

  1. Host buckets points by (i, j mod 4), capacity 128 per bucket.
     Bucket (i, phi) -> tile t = (i-1)*4 + phi; 509*4 = 2036 tiles/core.
  2. Device table T[i, cb, (q,k,c)] = grid[i-1+k, 4cb+q-1, c] (fp16,
     q in 0..7) built once from CP_locs. B1(i) = T[i] is a [128cb, 64]
     matmul operand shared by the 4 phi-tiles of row i.
  3. Per tile: V1[m, cb] = (cb == j_m div 4) via iota + is_equal (DVE),
     DMA-transposed (XBAR) to V1T[cb, m], then PE matmul
     out1[m, (q,k,c)] = V1T.T @ B1 gathers each point's 8-col band.
  4. Fine taps are compile-time slices q = phi + t; DVE contracts
     t (Wy), then k (Wx), then accumulates squared diffs.
  5. Bucket overflow (~4.7K pts/core) goes through one small dma_gather
     (8192 idx) against the same table, 2-row 256B elems, grouped by
     j mod 8; contracted with the same fine-tap code.

Host work stays permutation/padding/packing only; all arithmetic on
CP_locs / r / ch1 happens on device.
"""

import sys

for _p in ("/opt/trn_rl_repo",):
    if _p not in sys.path:
        sys.path.insert(0, _p)

from contextlib import ExitStack

import numpy as np

from concourse import bacc, bass, mybir, tile
from concourse.bass_utils import run_bass_kernel_spmd

F32 = mybir.dt.float32
F16 = mybir.dt.float16
I16 = mybir.dt.int16
I32 = mybir.dt.int32
OP = mybir.AluOpType

G = 512
N_CORES = 8
NI = 509                 # i values 1..509
NT = NI * 4              # main tiles per core
CAP = 128                # points per bucket
CH_I = 32                # i rows per chunk
N_CH = (NI + CH_I - 1) // CH_I
S_SP = 64                # spill slots (8 segments x 8 slots x 128 pts)
SP_PTS = S_SP * 128      # 8192
SEG = 8                  # slots per phase8 segment
NF = NT + S_SP           # stream width


def build_nc(stages=99):
    nc = bacc.Bacc("TRN2", target_bir_lowering=False, debug=False,
                   dynamic_dma_scratch_size=72704)

    cp = nc.dram_tensor("cp", [G, G, 2], F32, kind="ExternalInput")
    xs = nc.dram_tensor("xs", [128, NF], F32, kind="ExternalInput")
    ys = nc.dram_tensor("ys", [128, NF], F32, kind="ExternalInput")
    cbt = nc.dram_tensor("cbt", [NT, 128], mybir.dt.int8,
                         kind="ExternalInput")
    c01 = nc.dram_tensor("c01", [128, NF, 2], F16, kind="ExternalInput")
    sidx = nc.dram_tensor("sidx", [128, SP_PTS // 16], I16,
                          kind="ExternalInput")
    out = nc.dram_tensor("out", [128, 1], F32, kind="ExternalOutput")

    # padded grid: row_pad = i + 1 (rows -1..514), col_pad = j + 1
    cpp = nc.dram_tensor("cpp", [516, 1040], F32)
    # table: [i, cb, (q, k, c)] fp16
    tbl = nc.dram_tensor("tbl", [512, 8192], F16)
    tbl_3d = tbl.ap().rearrange("i (cb e) -> i cb e", e=64)
    tbl_flat = tbl.ap().rearrange("i (f e) -> (i f) e", e=128)
    cp_ap = cp.ap().rearrange("a b c -> a (b c)")       # [512, 1024]

    with tile.TileContext(nc) as tc, ExitStack() as ctx:
        wt_pool = ctx.enter_context(tc.tile_pool(name="wt", bufs=1))

        # ---- long-lived stream tiles ------------------------------------
        xs_t = wt_pool.tile([128, NF], F32, name="xs_t")
        ys_t = wt_pool.tile([128, NF], F32, name="ys_t")
        c01_t = wt_pool.tile([128, NF, 2], F16, name="c01_t")
        sidx_t = wt_pool.tile([128, SP_PTS // 16], I16, name="sidx_t")
        iota8 = wt_pool.tile([128, 1], mybir.dt.int8, name="iota8")
        wy = [wt_pool.tile([128, NF], F16, name=f"wy{t}") for t in range(4)]
        wxi = wt_pool.tile([128, NF, 4], F16, name="wxi")
        acc = wt_pool.tile([128, 4 * CH_I * 2], F32, name="acc")
        acc_sp = wt_pool.tile([128, 8 * SEG * 2], F32, name="acc_sp")
        acc4 = acc[:].rearrange("p (a b c) -> p a b c", b=CH_I, c=2)
        acc_sp4 = acc_sp[:].rearrange("p (a b c) -> p a b c", b=SEG, c=2)
        g_sp = wt_pool.tile([128, S_SP, 128], F16, name="g_sp")

        nc.sync.dma_start(out=xs_t[:], in_=xs.ap()[:, :])
        nc.sync.dma_start(out=ys_t[:], in_=ys.ap()[:, :])
        nc.sync.dma_start(out=c01_t[:], in_=c01.ap()[:, :, :])
        nc.sync.dma_start(out=sidx_t[:], in_=sidx.ap()[:, :])
        nc.vector.memset(acc[:], 0.0)
        nc.vector.memset(acc_sp[:], 0.0)

        with ExitStack() as bctx:
            const_pool = bctx.enter_context(tc.tile_pool(name="const", bufs=1))
            band_pool = bctx.enter_context(tc.tile_pool(name="band", bufs=1))
            sin_pool = bctx.enter_context(tc.tile_pool(name="sin", bufs=2))
            tmp_pool = bctx.enter_context(tc.tile_pool(name="tmp", bufs=1))

            # ---- phase 0: padded grid copy ------------------------------
            z = const_pool.tile([128, 1040], F32)
            nc.vector.memset(z[:], 0.0)
            for r0 in range(0, 516, 128):
                r1 = min(r0 + 128, 516)
                nc.sync.dma_start(out=cpp.ap()[r0:r1, :], in_=z[: r1 - r0, :])
            nc.sync.dma_start(out=cpp.ap()[1:513, 2:1026], in_=cp_ap[:, :])

            # ---- phase A: build table -----------------------------------
            # T[i, cb, q, k, c] = cpp[i+k, 8cb+2q+c] (f32 elem index)
            for b in range(4 if stages >= 1 else 0):
                t_band = band_pool.tile([128, 8192], F16, tag="tband")
                dst4 = t_band[:].rearrange("p (cb q e) -> p cb q e", q=8, e=8)
                for k in range(4):
                    s_in = sin_pool.tile([128, 1040], F32, tag="sin")
                    nc.sync.dma_start(
                        out=s_in[:], in_=cpp.ap()[128 * b + k: 128 * b + k + 128, :])
                    # f32 elem for (cb, q, c) is 8cb + 2q + c; the q-window
                    # straddles 8-blocks, so slice from offset 4*qq first
                    for qq in range(4):      # q pair (2qq, 2qq+1)
                        src = s_in[:, 4 * qq: 4 * qq + 1024].rearrange(
                            "p (cb h) -> p cb h", h=8)[:, :, 0:4].rearrange(
                            "p cb (a c) -> p cb a c", c=2)
                        nc.vector.tensor_copy(
                            out=dst4[:, :, 2 * qq: 2 * qq + 2,
                                     2 * k: 2 * k + 2],
                            in_=src,
                        )
                nc.sync.dma_start(out=tbl.ap()[128 * b:128 * b + 128, :],
                                  in_=t_band[:])

            # ---- phase B: weights + iota --------------------------------
            io32 = tmp_pool.tile([128, 1], I32, tag="io32")
            nc.gpsimd.iota(io32[:], pattern=[[1, 1]], base=0,
                           channel_multiplier=1)
            nc.vector.tensor_copy(out=iota8[:], in_=io32[:])

            def weights(v_t, targets):
                ta = tmp_pool.tile([128, NF], F32, tag="ta", name="ta")
                tb = tmp_pool.tile([128, NF], F32, tag="tb", name="tb")
                tc_ = tmp_pool.tile([128, NF], F32, tag="tc", name="tc_")
                td = tmp_pool.tile([128, NF], F32, tag="td", name="td")
                V = v_t[:]
                W = targets
                nc.vector.tensor_scalar(ta[:], V, -1.0, None, OP.add)
                nc.vector.tensor_tensor(tb[:], V, V, OP.mult)
                nc.vector.tensor_tensor(tc_[:], ta[:], ta[:], OP.mult)
                # w0 = -0.5*x*(x-1)^2 ; w3 = 0.5*x^2*(x-1)
                nc.vector.scalar_tensor_tensor(W[0], V, -0.5, tc_[:],
                                               OP.mult, OP.mult)
                nc.vector.scalar_tensor_tensor(W[3], tb[:], 0.5, ta[:],
                                               OP.mult, OP.mult)
                # w1 = (1.5x - 2.5)*x^2 + 1
                nc.vector.tensor_scalar(td[:], V, 1.5, -2.5, OP.mult, OP.add)
                tc2 = tmp_pool.tile([128, NF], F32, tag="tc", name="tc2")
                nc.vector.tensor_tensor(tc2[:], td[:], tb[:], OP.mult)
                nc.vector.tensor_scalar(W[1], tc2[:], 1.0, None, OP.add)
                # w2 = 1 - w0 - w1 - w3
                ta2 = tmp_pool.tile([128, NF], F32, tag="ta", name="ta2")
                nc.vector.tensor_tensor(ta2[:], W[1], W[0], OP.add)
                tb2 = tmp_pool.tile([128, NF], F32, tag="tb", name="tb2")
                nc.vector.tensor_tensor(tb2[:], ta2[:], W[3], OP.add)
                nc.vector.tensor_scalar(W[2], tb2[:], -1.0, 1.0, OP.mult,
                                        OP.add)

            if stages >= 2:
                weights(ys_t, [w[:] for w in wy])
                weights(xs_t, [wxi[:, :, k] for k in range(4)])

        # ---- main loop pools --------------------------------------------
        cbr_pool = ctx.enter_context(tc.tile_pool(name="cbr", bufs=3))
        v1t_pool = ctx.enter_context(tc.tile_pool(name="v1t", bufs=4))
        b1_pool = ctx.enter_context(tc.tile_pool(name="b1", bufs=3))
        po_pool = ctx.enter_context(tc.psum_pool(name="po", bufs=4))
        o1_pool = ctx.enter_context(tc.tile_pool(name="o1", bufs=2))
        f_pool = ctx.enter_context(tc.tile_pool(name="f", bufs=2))

        iota8_b = iota8[:].rearrange("p (a f) -> p a f", a=1)
        cbt_flat = cbt.ap().rearrange("t m -> (t m)")

        def emit_v1t(ch, g):
            # V1T sub-slab for tiles [ch*128 + 16g, +16)
            nI = min(CH_I, NI - ch * CH_I)
            t0 = ch * 4 * CH_I + 16 * g
            n = min(16, 4 * nI - 16 * g)
            if n <= 0:
                return None
            cbr = cbr_pool.tile([128, 16, 128], mybir.dt.int8, tag="cbr")
            src = cbt_flat[t0 * 128:(t0 + n) * 128].rearrange(
                "(a f) -> a f", a=1).to_broadcast([128, n * 128])
            nc.sync.dma_start(
                out=cbr[:, :n, :].rearrange("p a b -> p (a b)"), in_=src)
            v1t = v1t_pool.tile([128, 16, 128], F16, tag="v1t")
            nc.vector.tensor_tensor(
                v1t[:, :n, :], cbr[:, :n, :],
                iota8_b.to_broadcast([128, n, 128]),
                OP.is_equal)
            return v1t

        def emit_mm(ch, g, v1t, o1):
            nI = min(CH_I, NI - ch * CH_I)
            n = min(16, 4 * nI - 16 * g)
            if n <= 0:
                return
            for ii in range(n // 4):      # i rows in this sub-slab
                iloc = 4 * g + ii
                i_glob = 1 + ch * CH_I + iloc
                b1 = b1_pool.tile([128, 64], F16, tag="b1")
                nc.sync.dma_start(out=b1[:], in_=tbl_3d[i_glob])
                po = po_pool.tile([128, 4, 64], F32, tag="po")
                for phi in range(4):
                    nc.tensor.matmul(out=po[:, phi, :],
                                     lhsT=v1t[:, 4 * ii + phi, :],
                                     rhs=b1[:], start=True, stop=True)
                nc.scalar.copy(out=o1[:, 4 * iloc:4 * iloc + 4, :],
                               in_=po[:])

        def fine(o1_ap, nI, wy_sl, wxi_sl, c01_sl, acc_sl, offs):
            """Contract taps. o1_ap [128, nI, 64]; offs[t] = byte.. elem
            offset of 8-wide (k,c) slice for tap t. wy_sl(t) -> [128, nI],
            wxi_sl -> [128, nI, 4], c01_sl -> [128, nI, 2], acc_sl [128,nI,2]."""
            t2 = f_pool.tile([128, nI, 8], F16, tag="t2")
            rt = f_pool.tile([128, nI, 8], F16, tag="rt")
            for t in range(4):
                src = o1_ap[:, :, offs[t]:offs[t] + 8]
                wb = wy_sl(t).to_broadcast([128, nI, 8])
                if t == 0:
                    nc.vector.tensor_tensor(t2[:], src, wb, OP.mult)
                else:
                    nc.vector.tensor_tensor(rt[:], src, wb, OP.mult)
                    nc.vector.tensor_tensor(t2[:], t2[:], rt[:], OP.add)
            p4 = f_pool.tile([128, nI, 4, 2], F16, tag="p4")
            nc.gpsimd.tensor_tensor(
                p4[:], t2[:].rearrange("p i (k c) -> p i k c", c=2),
                wxi_sl.to_broadcast([128, nI, 4, 2]), OP.mult)
            m01 = f_pool.tile([128, nI, 2], F16, tag="m01")
            m23 = f_pool.tile([128, nI, 2], F16, tag="m23")
            mp = f_pool.tile([128, nI, 2], F16, tag="mp")
            nc.gpsimd.tensor_tensor(m01[:], p4[:, :, 0, :], p4[:, :, 1, :],
                                    OP.add)
            nc.gpsimd.tensor_tensor(m23[:], p4[:, :, 2, :], p4[:, :, 3, :],
                                    OP.add)
            nc.gpsimd.tensor_tensor(mp[:], m01[:], m23[:], OP.add)
            d = f_pool.tile([128, nI, 2], F32, tag="d")
            sq = f_pool.tile([128, nI, 2], F32, tag="sq")
            nc.vector.tensor_tensor(d[:], mp[:], c01_sl, OP.subtract)
            nc.vector.tensor_tensor(sq[:], d[:], d[:], OP.mult)
            nc.vector.tensor_tensor(acc_sl, acc_sl, sq[:], OP.add)

        def fine_chunk(ch, o1):
            nI = min(CH_I, NI - ch * CH_I)
            t0 = ch * 4 * CH_I
            o4 = o1[:].rearrange("p (i f) q -> p i f q", f=4)
            for phi in range(4):
                def wy_sl(t, phi=phi):
                    return wy[t][:, t0:t0 + 4 * nI].rearrange(
                        "p (i f) -> p i f", f=4)[:, :, phi]
                wxi_sl = wxi[:, t0:t0 + 4 * nI, :].rearrange(
                    "p (i f) k -> p i f k", f=4)[:, :, phi, :]
                c01_sl = c01_t[:, t0:t0 + 4 * nI, :].rearrange(
                    "p (i f) c -> p i f c", f=4)[:, :, phi, :]
                fine(o4[:, :nI, phi, :], nI, wy_sl, wxi_sl, c01_sl,
                     acc4[:, phi, :nI, :],
                     [(phi + t) * 8 for t in range(4)])

        def fine_spill():
            for p8 in range(8):
                base = p8 * 8 + (32 if p8 >= 4 else 0)
                c0 = NT + p8 * SEG
                def wy_sl(t, c0=c0):
                    return wy[t][:, c0:c0 + SEG]
                fine(g_sp[:, p8 * SEG:(p8 + 1) * SEG, :], SEG, wy_sl,
                     wxi[:, c0:c0 + SEG, :],
                     c01_t[:, c0:c0 + SEG, :],
                     acc_sp4[:, p8, :, :],
                     [base + t * 8 for t in range(4)])

        # ---- main loop: software-pipelined emission ---------------------
        if stages >= 3:
            o1_tiles = {}
            o1_tiles[0] = o1_pool.tile([128, 4 * CH_I, 64], F16, tag="o1",
                                       name="o1_0")
            for g in range(8):
                v1t = emit_v1t(0, g)
                emit_mm(0, g, v1t, o1_tiles[0])
            for ch in range(1, N_CH):
                o1_tiles[ch] = o1_pool.tile([128, 4 * CH_I, 64], F16,
                                            tag="o1", name=f"o1_{ch}")
                for g in range(8):
                    v1t = emit_v1t(ch, g)
                    if v1t is None:
                        continue
                    emit_mm(ch, g, v1t, o1_tiles[ch])
                    if g == 2:
                        if ch == 1:
                            # spill gather: emitted late so the table
                            # writes have long since drained
                            nc.gpsimd.dma_gather(
                                g_sp[:], tbl_flat, sidx_t[:], SP_PTS,
                                SP_PTS, 128, single_packet=False)
                        fine_chunk(ch - 1, o1_tiles[ch - 1])
                        del o1_tiles[ch - 1]
            fine_chunk(N_CH - 1, o1_tiles[N_CH - 1])
            if stages >= 4:
                fine_spill()

        # ---- reduce -----------------------------------------------------
        r1 = wt_pool.tile([128, 1], F32, name="r1")
        r2 = wt_pool.tile([128, 1], F32, name="r2")
        rs = wt_pool.tile([128, 1], F32, name="rs")
        nc.vector.tensor_reduce(r1[:], acc[:], mybir.AxisListType.X, OP.add)
        nc.vector.tensor_reduce(r2[:], acc_sp[:], mybir.AxisListType.X,
                                OP.add)
        nc.vector.tensor_tensor(rs[:], r1[:], r2[:], OP.add)
        nc.sync.dma_start(out=out.ap()[:, :], in_=rs[:])

    nc.compile()
    return nc


def host_prep(ch1, CP_locs, CP_idx, r, n_cores=N_CORES):
    N = ch1.shape[0]
    per = N // n_cores
    assert per * n_cores == N
    cp_f = np.ascontiguousarray(CP_locs, dtype=np.float32)

    # dummy defaults for main tiles: point (i, 4+phi), x=y=0
    t_arr = np.arange(NT)
    di = 1 + t_arr // 4
    dj = 4 + (t_arr % 4)
    c01_dummy_main = cp_f[di, dj]                     # [NT, 2]
    s_arr = np.arange(S_SP)
    c01_dummy_sp = cp_f[1, 8 + s_arr // SEG]          # [S_SP, 2]

    in_maps = []
    for c in range(n_cores):
        sl = slice(c * per, (c + 1) * per)
        i = CP_idx[sl, 0].astype(np.int64)
        j = CP_idx[sl, 1].astype(np.int64)
        x = r[sl, 0].astype(np.float32) % 1.0
        y = r[sl, 1].astype(np.float32) % 1.0
        c1 = ch1[sl].astype(np.float32)

        key = (i - 1) * 4 + (j & 3)
        order = np.argsort(key, kind="stable")
        ks = key[order]
        starts = np.searchsorted(ks, np.arange(NT), side="left")
        rank = np.arange(per) - starts[ks]            # rank within bucket

        xs_a = np.zeros((128, NF), np.float32)
        ys_a = np.zeros((128, NF), np.float32)
        cbt_a = np.full((NT, 128), 1, np.int8)
        c01_a = np.empty((128, NF, 2), np.float16)
        c01_a[:, :NT] = c01_dummy_main[None, :, :]
        c01_a[:, NT:] = c01_dummy_sp[None, :, :]
        sg = np.full(SP_PTS, 65, np.int16)

        main = rank < CAP
        mo = order[main]
        p_m = rank[main]
        t_m = ks[main]
        xs_a[p_m, t_m] = x[mo]
        ys_a[p_m, t_m] = y[mo]
        cbt_a[t_m, p_m] = (j[mo] >> 2).astype(np.int8)
        c01_a[p_m, t_m] = c1[mo]

        so = order[~main]                             # spill, bucket-sorted
        ph8 = (j[so] & 7).astype(np.int64)
        so = so[np.argsort(ph8, kind="stable")]
        ph8 = (j[so] & 7).astype(np.int64)
        st8 = np.searchsorted(ph8, np.arange(8), side="left")
        rank8 = np.arange(len(so)) - st8[ph8]
        assert (rank8 < SEG * 128).all(), \
            f"spill segment overflow: {rank8.max()}"
        gpos = ph8 * (SEG * 128) + rank8
        p_s = gpos % 128
        col = NT + gpos // 128
        xs_a[p_s, col] = x[so]
        ys_a[p_s, col] = y[so]
        c01_a[p_s, col] = c1[so]
        sg[gpos] = (i[so] * 64 + (j[so] >> 3)).astype(np.int16)

        sidx_a = np.tile(sg.reshape(SP_PTS // 16, 16).T, (8, 1))

        in_maps.append({
            "cp": cp_f, "xs": xs_a, "ys": ys_a, "cbt": cbt_a,
            "c01": np.ascontiguousarray(c01_a), "sidx": sidx_a,
        })
    return in_maps


_NC_CACHE = {}


def kernel(ch1, CP_locs, CP_idx, r):
    ch1, CP_locs = np.asarray(ch1), np.asarray(CP_locs)
    CP_idx, r = np.asarray(CP_idx), np.asarray(r)
    if "nc" not in _NC_CACHE:
        _NC_CACHE["nc"] = build_nc()
    nc = _NC_CACHE["nc"]
    in_maps = host_prep(ch1, CP_locs, CP_idx, r)
    res = run_bass_kernel_spmd(nc, in_maps, list(range(N_CORES)))
    total = np.float64(0.0)
    for rmap in res.results:
        total += np.float64(rmap["out"]).sum()
    return np.array(total, dtype=np.float32)
